# revision 13
# baseline (speedup 1.0000x reference)
"""Trainium2 Bass kernel for nn_FreqCrossAttention — radix-2 DIT version.

Sharding: 8 cores = 4 batches x 2 head-groups (8 heads each). Each core
computes a partial output [2048, 1024] (its head-group's contribution
through W_o row-parallel); host sums the pair per batch.

Freq row layout (1025 rows, used for q rows, kv rows and attention):
  row r in [0, 512)    -> freq k = r          (lo block)
  row r in [512, 1025) -> freq k = 1536 - r   (hi block, stored by m=r-512)
rfft via radix-2 DIT: A = rDFT_1024(even samples), B = twiddle-folded
rDFT_1024(odd samples); Xlo = A+B, Xhi = conj(A-B). irfft via
u = lo+conj(hi), p = lo-conj(hi) and twiddle-folded G matrices.

All matmuls bf16. Everything stays in SBUF between phases (no DRAM
round-trips), keeping the PE HAM clock-gate warm.
"""
import math
import numpy as np
import ml_dtypes

B, L, E, H = 4, 2048, 1024, 16
D = E // H            # 64
NH = 8                # heads per core
P = 128
Lh = L // 2           # 1024
Fh = Lh // 2 + 1      # 513
NF = 1025             # freq rows per side
ET = 8                # e-blocks
EPS = 1e-5
SQL = math.sqrt(L)
# freq row tiles: 8 x 128 + 1
FT = [(i * P, P) for i in range(8)] + [(1024, 1)]
# proj f-chunks: (src block, offset, size, global dst offset)
PCH = [("lo", 0, 512, 0), ("hi", 0, 256, 512), ("hi", 256, 257, 768)]
# scores f-chunks over global rows
SCH = [(0, 512), (512, 256), (768, 257)]

_CACHE = {}


def _consts():
    NORM = 1.0 / math.sqrt(L)
    n = np.arange(Lh)[:, None]
    m = np.arange(Fh)[None, :]
    ang_e = 2 * np.pi * n * m / Lh
    FAc = (np.cos(ang_e) * NORM).astype(np.float32)
    FAs = (-np.sin(ang_e) * NORM).astype(np.float32)
    ang_o = 2 * np.pi * m * (2 * n + 1) / L
    FBc = (np.cos(ang_o) * NORM).astype(np.float32)
    FBs = (-np.sin(ang_o) * NORM).astype(np.float32)

    k = np.arange(Fh)[:, None]
    nn_ = np.arange(Lh)[None, :]
    ck = np.where((k == 0) | (k == Lh // 2), 1.0, 2.0)
    th = 2 * np.pi * k * nn_ / Lh
    GAc = (ck * np.cos(th) * NORM).astype(np.float32)
    GAs = (-ck * np.sin(th) * NORM).astype(np.float32)
    ph = th + np.pi * k / Lh
    GBc = (ck * np.cos(ph) * NORM).astype(np.float32)
    GBs = (-ck * np.sin(ph) * NORM).astype(np.float32)
    GAc[512] *= 2.0   # row-512 fold: u[512] = 2*Re(lo[512]), fed as our[8]
    GBs[512] *= 2.0   # p[512] = 2i*Im(lo[512]), fed as oui[8]
    return FAc, FAs, FBc, FBs, GAc, GAs, GBc, GBs


def _build():
    import concourse.bass as bass
    import concourse.bacc as bacc
    import concourse.mybir as mybir
    import concourse.tile as tile

    R = mybir.dt.bfloat16
    F32 = mybir.dt.float32
    AF = mybir.ActivationFunctionType
    AL = mybir.AluOpType

    nc = bacc.Bacc("TRN2", debug=False, num_devices=8)

    q_d = nc.dram_tensor("q", [L, E], R, kind="ExternalInput")
    kv_d = nc.dram_tensor("kv", [L, E], R, kind="ExternalInput")
    FAc_d = nc.dram_tensor("FAc", [Lh, Fh], R, kind="ExternalInput")
    FAs_d = nc.dram_tensor("FAs", [Lh, Fh], R, kind="ExternalInput")
    FBc_d = nc.dram_tensor("FBc", [Lh, Fh], R, kind="ExternalInput")
    FBs_d = nc.dram_tensor("FBs", [Lh, Fh], R, kind="ExternalInput")
    GAc_d = nc.dram_tensor("GAc", [Fh, Lh], R, kind="ExternalInput")
    GAs_d = nc.dram_tensor("GAs", [Fh, Lh], R, kind="ExternalInput")
    GBc_d = nc.dram_tensor("GBc", [Fh, Lh], R, kind="ExternalInput")
    GBs_d = nc.dram_tensor("GBs", [Fh, Lh], R, kind="ExternalInput")
    W_d = {}
    for nm in ("qr", "qi", "kr", "ki", "vr", "vi"):
        W_d[nm] = nc.dram_tensor(f"W{nm}", [E, 512], R, kind="ExternalInput")
        W_d["b" + nm] = nc.dram_tensor(f"b{nm}", [512, 1], F32, kind="ExternalInput")
    qdc_d = nc.dram_tensor("qdc", [P, NH], F32, kind="ExternalInput")
    WoT_d = nc.dram_tensor("WoT", [512, E], R, kind="ExternalInput")
    out_d = nc.dram_tensor("out", [L, E], R, kind="ExternalOutput")

    with tile.TileContext(nc) as tc:
      with tc.tile_pool(name="persist", bufs=1) as persist:
        eps_t = persist.tile([P, 1], F32)
        nc.vector.memset(eps_t[:], EPS)
        bias_t = {}
        for nm in ("qr", "qi", "kr", "ki"):
            bias_t[nm] = []
            for mt in range(4):
                bt_ = persist.tile([P, 1], F32, tag=f"b{nm}{mt}", name=f"b{nm}{mt}")
                nc.sync.dma_start(bt_[:], W_d["b" + nm].ap()[mt * P:(mt + 1) * P, :])
                bias_t[nm].append(bt_)
        qdct = persist.tile([P, NH], F32)
        nc.sync.dma_start(qdct[:], qdc_d.ap())
        # V bias broadcast tiles
        vb_row = persist.tile([1, 512], F32)
        vbias = {}
        for nm in ("vr", "vi"):
            nc.sync.dma_start(vb_row[:], W_d["b" + nm].ap().rearrange("e one -> one e"))
            vb = persist.tile([P, 512], F32, tag=f"vb{nm}", name=f"vb{nm}")
            nc.gpsimd.partition_broadcast(vb[:], vb_row[:])
            vbias[nm] = vb

        # cat pool: attention operands (live until end of attention)
        with tc.tile_pool(name="cat", bufs=1) as catp:
            Qc = [catp.tile([P, NF], R, tag=f"Qc{h}", name=f"Qc{h}") for h in range(NH)]
            Kc = [catp.tile([P, NF], R, tag=f"Kc{h}", name=f"Kc{h}") for h in range(NH)]
            Vc = [catp.tile([P, NH * 129], R, tag=f"Vc{i}", name=f"Vc{i}") for i in range(9)]

            Wt = {}

            def dft_path(xE, xO, fa_c, fa_s, fb_c, fb_s,
                         xlo_r, xlo_i, xhi_r, xhi_i, pfx):
                with tc.tile_pool(name=f"dps{pfx}", bufs=1, space="PSUM") as dps, \
                     tc.tile_pool(name=f"sev{pfx}", bufs=2) as sev:
                    for eb in range(ET):
                        # one accumulation group per PSUM bank (start=True
                        # clears has_written for the WHOLE bank);
                        # cos chains first so the first MMs need less DMA
                        paR = dps.tile([P, 256], F32, tag="paR", name="paR")
                        paI = dps.tile([P, 256], F32, tag="paI", name="paI")
                        pa2r = dps.tile([P, 257], F32, tag="pa2r", name="pa2r")
                        pa2i = dps.tile([P, 257], F32, tag="pa2i", name="pa2i")
                        for t in range(8):
                            lhs = xE[t][:, eb * P:(eb + 1) * P]
                            st_, sp_ = (t == 0), (t == 7)
                            nc.tensor.matmul(paR[:, 0:256], lhs, fa_c[t][:, 0:256],
                                             start=st_, stop=sp_)
                            nc.tensor.matmul(pa2r[:, 0:257], lhs, fa_c[t][:, 256:513],
                                             start=st_, stop=sp_)
                        for t in range(8):
                            lhs = xE[t][:, eb * P:(eb + 1) * P]
                            st_, sp_ = (t == 0), (t == 7)
                            nc.tensor.matmul(paI[:, 0:256], lhs, fa_s[t][:, 0:256],
                                             start=st_, stop=sp_)
                            nc.tensor.matmul(pa2i[:, 0:257], lhs, fa_s[t][:, 256:513],
                                             start=st_, stop=sp_)
                        sAr = sev.tile([P, Fh], F32, tag="sAr", name="sAr", bufs=2)
                        sAi = sev.tile([P, Fh], F32, tag="sAi", name="sAi", bufs=2)
                        nc.scalar.copy(sAr[:, 0:256], paR[:, 0:256])
                        nc.scalar.copy(sAr[:, 256:513], pa2r[:, 0:257])
                        nc.scalar.copy(sAi[:, 0:256], paI[:, 0:256])
                        nc.scalar.copy(sAi[:, 256:513], pa2i[:, 0:257])
                        pbR = dps.tile([P, 256], F32, tag="pbR", name="pbR")
                        pbI = dps.tile([P, 256], F32, tag="pbI", name="pbI")
                        pb2r = dps.tile([P, 257], F32, tag="pb2r", name="pb2r")
                        pb2i = dps.tile([P, 257], F32, tag="pb2i", name="pb2i")
                        for t in range(8):
                            lhs = xO[t][:, eb * P:(eb + 1) * P]
                            st_, sp_ = (t == 0), (t == 7)
                            nc.tensor.matmul(pbR[:, 0:256], lhs, fb_c[t][:, 0:256],
                                             start=st_, stop=sp_)
                            nc.tensor.matmul(pb2r[:, 0:257], lhs, fb_c[t][:, 256:513],
                                             start=st_, stop=sp_)
                        for t in range(8):
                            lhs = xO[t][:, eb * P:(eb + 1) * P]
                            st_, sp_ = (t == 0), (t == 7)
                            nc.tensor.matmul(pbI[:, 0:256], lhs, fb_s[t][:, 0:256],
                                             start=st_, stop=sp_)
                            nc.tensor.matmul(pb2i[:, 0:257], lhs, fb_s[t][:, 256:513],
                                             start=st_, stop=sp_)
                        # combines (vector engine; one PSUM operand max)
                        nc.vector.tensor_tensor(
                            xlo_r[eb][:, 0:256], sAr[:, 0:256], pbR[:, 0:256], op=AL.add)
                        nc.vector.tensor_tensor(
                            xlo_r[eb][:, 256:512], sAr[:, 256:512], pb2r[:, 0:256], op=AL.add)
                        nc.vector.tensor_tensor(
                            xlo_i[eb][:, 0:256], sAi[:, 0:256], pbI[:, 0:256], op=AL.add)
                        nc.vector.tensor_tensor(
                            xlo_i[eb][:, 256:512], sAi[:, 256:512], pb2i[:, 0:256], op=AL.add)
                        nc.vector.tensor_tensor(
                            xhi_r[eb][:, 0:256], sAr[:, 0:256], pbR[:, 0:256], op=AL.subtract)
                        nc.vector.tensor_tensor(
                            xhi_r[eb][:, 256:513], sAr[:, 256:513], pb2r[:, 0:257], op=AL.subtract)
                        nc.vector.tensor_tensor(
                            xhi_i[eb][:, 0:256], pbI[:, 0:256], sAi[:, 0:256], op=AL.subtract)
                        nc.vector.tensor_tensor(
                            xhi_i[eb][:, 256:513], pb2i[:, 0:257], sAi[:, 256:513], op=AL.subtract)

            def qk_proj(xlo_r, xlo_i, xhi_r, xhi_i, nmr, nmi, cat, qdc_fix):
                with tc.tile_pool(name="pps", bufs=1, space="PSUM") as pps, \
                     tc.tile_pool(name="stg", bufs=2) as stg:
                    for (src, c0, csz, g0) in PCH:
                        xr = xlo_r if src == "lo" else xhi_r
                        xi = xlo_i if src == "lo" else xhi_i
                        for mt in range(4):
                            ppr = pps.tile([P, 512], F32, tag="ppr", name="ppr", bufs=2)
                            ppi = pps.tile([P, 512], F32, tag="ppi", name="ppi", bufs=2)
                            for ec in range(ET):
                                st_, sp_ = (ec == 0), (ec == ET - 1)
                                nc.tensor.matmul(ppr[:, 0:csz],
                                                 Wt[nmr][ec][:, mt * P:(mt + 1) * P],
                                                 xr[ec][:, c0:c0 + csz],
                                                 start=st_, stop=sp_)
                                nc.tensor.matmul(ppi[:, 0:csz],
                                                 Wt[nmi][ec][:, mt * P:(mt + 1) * P],
                                                 xi[ec][:, c0:c0 + csz],
                                                 start=st_, stop=sp_)
                            sgr = stg.tile([P, 512], R, tag="sgr", name="sgr")
                            sgi = stg.tile([P, 512], R, tag="sgi", name="sgi")
                            nc.scalar.activation(sgr[:, 0:csz], ppr[:, 0:csz],
                                                 AF.Identity, bias=bias_t[nmr][mt][:])
                            nc.scalar.activation(sgi[:, 0:csz], ppi[:, 0:csz],
                                                 AF.Identity, bias=bias_t[nmi][mt][:])
                            h0, h1 = 2 * mt, 2 * mt + 1
                            nc.sync.dma_start(cat[h0][0:64, g0:g0 + csz], sgr[0:64, 0:csz])
                            nc.sync.dma_start(cat[h1][0:64, g0:g0 + csz], sgr[64:128, 0:csz])
                            nc.sync.dma_start(cat[h0][64:128, g0:g0 + csz], sgi[0:64, 0:csz])
                            nc.sync.dma_start(cat[h1][64:128, g0:g0 + csz], sgi[64:128, 0:csz])
                    if qdc_fix:
                        for h in range(NH):
                            nc.vector.tensor_tensor(cat[h][:, 0:1], cat[h][:, 0:1],
                                                    qdct[:, h:h + 1], op=AL.add)

            def v_proj(xlo_r, xlo_i, xhi_r, xhi_i):
                with tc.tile_pool(name="vps", bufs=1, space="PSUM") as vps:
                    for mi, (m0, msz) in enumerate(FT):
                        if mi < 4:
                            xr = [xlo_r[ec][:, m0:m0 + msz] for ec in range(ET)]
                            xi = [xlo_i[ec][:, m0:m0 + msz] for ec in range(ET)]
                        else:
                            c0 = m0 - 512
                            xr = [xhi_r[ec][:, c0:c0 + msz] for ec in range(ET)]
                            xi = [xhi_i[ec][:, c0:c0 + msz] for ec in range(ET)]
                        pvr = vps.tile([P, 512], F32, tag="pvr", name="pvr", bufs=2)
                        pvi = vps.tile([P, 512], F32, tag="pvi", name="pvi", bufs=2)
                        for ec in range(ET):
                            st_, sp_ = (ec == 0), (ec == ET - 1)
                            nc.tensor.matmul(pvr[0:msz, :], xr[ec], Wt["vr"][ec][:],
                                             start=st_, stop=sp_)
                            nc.tensor.matmul(pvi[0:msz, :], xi[ec], Wt["vi"][ec][:],
                                             start=st_, stop=sp_)
                        vco = Vc[mi][0:msz, :].rearrange("p (h c) -> p h c", h=NH)
                        nc.vector.tensor_add(
                            vco[:, :, 0:64],
                            pvr[0:msz, :].rearrange("p (h c) -> p h c", h=NH),
                            vbias["vr"][0:msz, :].rearrange("p (h c) -> p h c", h=NH))
                        nc.vector.tensor_add(
                            vco[:, :, 64:128],
                            pvi[0:msz, :].rearrange("p (h c) -> p h c", h=NH),
                            vbias["vi"][0:msz, :].rearrange("p (h c) -> p h c", h=NH))
                        nc.vector.memset(vco[:, :, 128:129], 1.0)

            # ---------------- DFT + projection phases ----------------
            with tc.tile_pool(name="fmat", bufs=1) as fm, \
                 tc.tile_pool(name="qn", bufs=1) as qnp:
                qnE = [qnp.tile([P, E], R, tag=f"qnE{t}", name=f"qnE{t}") for t in range(8)]
                qnO = [qnp.tile([P, E], R, tag=f"qnO{t}", name=f"qnO{t}") for t in range(8)]
                fa_c, fa_s, fb_c, fb_s = [], [], [], []

                # ---- kv path ----
                with tc.tile_pool(name="xkv", bufs=1) as xkv:
                    kXloR = [xkv.tile([P, 512], R, tag=f"kXloR{e}", name=f"kXloR{e}") for e in range(ET)]
                    kXloI = [xkv.tile([P, 512], R, tag=f"kXloI{e}", name=f"kXloI{e}") for e in range(ET)]
                    kXhiR = [xkv.tile([P, Fh], R, tag=f"kXhiR{e}", name=f"kXhiR{e}") for e in range(ET)]
                    kXhiI = [xkv.tile([P, Fh], R, tag=f"kXhiI{e}", name=f"kXhiI{e}") for e in range(ET)]
                    with tc.tile_pool(name="kvio", bufs=1) as kvio:
                        kvE, kvO = [], []
                        # issue in dependency-useful order: kvE+FAc+FAs first
                        for t in range(8):
                            kt = kvio.tile([P, E], R, tag=f"kvE{t}", name=f"kvE{t}")
                            nc.sync.dma_start(kt[:], kv_d.ap()[2 * P * t:2 * P * (t + 1):2, :])
                            kvE.append(kt)
                            fc = fm.tile([P, Fh], R, tag=f"fac{t}", name=f"fac{t}")
                            nc.sync.dma_start(fc[:], FAc_d.ap()[P * t:P * (t + 1), :])
                            fa_c.append(fc)
                            fs = fm.tile([P, Fh], R, tag=f"fas{t}", name=f"fas{t}")
                            nc.sync.dma_start(fs[:], FAs_d.ap()[P * t:P * (t + 1), :])
                            fa_s.append(fs)
                        for t in range(8):
                            kt = kvio.tile([P, E], R, tag=f"kvO{t}", name=f"kvO{t}")
                            nc.sync.dma_start(kt[:], kv_d.ap()[2 * P * t + 1:2 * P * (t + 1):2, :])
                            kvO.append(kt)
                            fc = fm.tile([P, Fh], R, tag=f"fbc{t}", name=f"fbc{t}")
                            nc.sync.dma_start(fc[:], FBc_d.ap()[P * t:P * (t + 1), :])
                            fb_c.append(fc)
                            fs = fm.tile([P, Fh], R, tag=f"fbs{t}", name=f"fbs{t}")
                            nc.sync.dma_start(fs[:], FBs_d.ap()[P * t:P * (t + 1), :])
                            fb_s.append(fs)

                        # LN of q -> qnE/qnO (scalar+vector, overlaps kv DFT)
                        lnio = kvio
                        with tc.tile_pool(name="lns", bufs=4) as lns:
                            for t in range(16):
                                par = t % 2
                                tt = t // 2
                                qt = lnio.tile([P, E], R, tag="qt", name="qt", bufs=3)
                                nc.sync.dma_start(
                                    qt[:], q_d.ap()[2 * P * tt + par:2 * P * (tt + 1):2, :])
                                st = lns.tile([P, 12], F32, tag="st", name="st")
                                nc.vector.bn_stats(st[:, 0:6], qt[:, 0:512])
                                nc.vector.bn_stats(st[:, 6:12], qt[:, 512:1024])
                                mv = lns.tile([P, 2], F32, tag="mv", name="mv")
                                nc.vector.bn_aggr(mv[:], st[:])
                                sd = lns.tile([P, 1], F32, tag="sd", name="sd")
                                nc.scalar.activation(sd[:], mv[:, 1:2], AF.Sqrt, bias=eps_t[:])
                                istd = lns.tile([P, 1], F32, tag="istd", name="istd")
                                nc.vector.reciprocal(istd[:], sd[:])
                                nmu = lns.tile([P, 1], F32, tag="nmu", name="nmu")
                                nc.vector.tensor_scalar_mul(nmu[:], mv[:, 0:1], -1.0)
                                nc.vector.tensor_mul(nmu[:], nmu[:], istd[:])
                                dst = qnE[tt] if par == 0 else qnO[tt]
                                nc.scalar.activation(dst[:], qt[:], AF.Identity,
                                                     bias=nmu[:], scale=istd[:])

                        dft_path(kvE, kvO, fa_c, fa_s, fb_c, fb_s,
                                 kXloR, kXloI, kXhiR, kXhiI, "kv")
                    # kvio closed: kvE/kvO + q-in freed.
                    # q DFT immediately after kv DFT: one continuous
                    # tensor-engine stream, projections afterwards.
                    with tc.tile_pool(name="xq", bufs=1) as xq:
                        qXloR = [xq.tile([P, 512], R, tag=f"qXloR{e}", name=f"qXloR{e}") for e in range(ET)]
                        qXloI = [xq.tile([P, 512], R, tag=f"qXloI{e}", name=f"qXloI{e}") for e in range(ET)]
                        qXhiR = [xq.tile([P, Fh], R, tag=f"qXhiR{e}", name=f"qXhiR{e}") for e in range(ET)]
                        qXhiI = [xq.tile([P, Fh], R, tag=f"qXhiI{e}", name=f"qXhiI{e}") for e in range(ET)]
                        with tc.tile_pool(name="wk", bufs=1) as wk:
                            for nm in ("kr", "ki"):
                                Wt[nm] = []
                                for ec in range(ET):
                                    w = wk.tile([P, 512], R, tag=f"W{nm}{ec}", name=f"W{nm}{ec}")
                                    nc.sync.dma_start(w[:], W_d[nm].ap()[ec * P:(ec + 1) * P, :])
                                    Wt[nm].append(w)
                            qk_proj(kXloR, kXloI, kXhiR, kXhiI, "kr", "ki", Kc, False)
                        with tc.tile_pool(name="wv", bufs=1) as wv:
                            for nm in ("vr", "vi"):
                                Wt[nm] = []
                                for ec in range(ET):
                                    w = wv.tile([P, 512], R, tag=f"W{nm}{ec}", name=f"W{nm}{ec}")
                                    nc.sync.dma_start(w[:], W_d[nm].ap()[ec * P:(ec + 1) * P, :])
                                    Wt[nm].append(w)
                            v_proj(kXloR, kXloI, kXhiR, kXhiI)
                        dft_path(qnE, qnO, fa_c, fa_s, fb_c, fb_s,
                                 qXloR, qXloI, qXhiR, qXhiI, "q")
                        with tc.tile_pool(name="wq", bufs=1) as wq:
                            for nm in ("qr", "qi"):
                                Wt[nm] = []
                                for ec in range(ET):
                                    w = wq.tile([P, 512], R, tag=f"W{nm}{ec}", name=f"W{nm}{ec}")
                                    nc.sync.dma_start(w[:], W_d[nm].ap()[ec * P:(ec + 1) * P, :])
                                    Wt[nm].append(w)
                            qk_proj(qXloR, qXloI, qXhiR, qXhiI, "qr", "qi", Qc, True)

            # ---------------- attention ---------------- (fm/qn closed)
            with tc.tile_pool(name="oacc", bufs=1) as oacc, \
                 tc.tile_pool(name="gmat", bufs=1) as gm:
                our = [oacc.tile([P, 512], R, tag=f"our{i}", name=f"our{i}") for i in range(9)]
                oui = [oacc.tile([P, 512], R, tag=f"oui{i}", name=f"oui{i}") for i in range(9)]
                # preload iDFT matrices during attention
                gac = [gm.tile([P, Lh], R, tag=f"gac{t}", name=f"gac{t}") for t in range(4)]
                gas = [gm.tile([P, Lh], R, tag=f"gas{t}", name=f"gas{t}") for t in range(4)]
                gbc = [gm.tile([P, Lh], R, tag=f"gbc{t}", name=f"gbc{t}") for t in range(4)]
                gbs = [gm.tile([P, Lh], R, tag=f"gbs{t}", name=f"gbs{t}") for t in range(4)]
                gac4 = gm.tile([1, Lh], R, tag="gac4", name="gac4")
                gbs4 = gm.tile([1, Lh], R, tag="gbs4", name="gbs4")
                for t in range(4):
                    nc.sync.dma_start(gac[t][:], GAc_d.ap()[t * P:(t + 1) * P, :])
                    nc.sync.dma_start(gas[t][:], GAs_d.ap()[t * P:(t + 1) * P, :])
                    nc.sync.dma_start(gbc[t][:], GBc_d.ap()[t * P:(t + 1) * P, :])
                    nc.sync.dma_start(gbs[t][:], GBs_d.ap()[t * P:(t + 1) * P, :])
                nc.sync.dma_start(gac4[:], GAc_d.ap()[512:513, :])
                nc.sync.dma_start(gbs4[:], GBs_d.ap()[512:513, :])

                with tc.tile_pool(name="expp", bufs=3) as expp, \
                     tc.tile_pool(name="sps", bufs=4, space="PSUM") as sps, \
                     tc.tile_pool(name="avps", bufs=4, space="PSUM") as avps, \
                     tc.tile_pool(name="nrm", bufs=4) as nrm:
                    for h in range(NH):
                        expts = []
                        for mi, (m0, msz) in enumerate(FT):
                            et_ = expp.tile([P, NF], R, tag=f"exp{mi}", name=f"exp{mi}")
                            for (f0, fsz) in SCH:
                                ps = sps.tile([P, 512], F32, tag="sc", name="sc")
                                nc.tensor.matmul(ps[0:msz, 0:fsz], Kc[h][:, m0:m0 + msz],
                                                 Qc[h][:, f0:f0 + fsz], start=True, stop=True)
                                nc.scalar.activation(et_[0:msz, f0:f0 + fsz], ps[0:msz, 0:fsz],
                                                     AF.Exp, scale=float(D ** -0.5))
                            expts.append(et_)
                        for lt, (l0, lsz) in enumerate(FT):
                            ps = avps.tile([P, 129], F32, tag="av", name="av")
                            for mi, (m0, msz) in enumerate(FT):
                                nc.tensor.matmul(ps[0:lsz, :], expts[mi][0:msz, l0:l0 + lsz],
                                                 Vc[mi][0:msz, h * 129:(h + 1) * 129],
                                                 start=(mi == 0), stop=(mi == 8))
                            rcp = nrm.tile([P, 1], F32, tag="rcp", name="rcp")
                            nc.vector.reciprocal(rcp[0:lsz, :], ps[0:lsz, 128:129])
                            nc.vector.tensor_scalar_mul(our[lt][0:lsz, h * 64:(h + 1) * 64],
                                                        ps[0:lsz, 0:64], rcp[0:lsz, :])
                            nc.vector.tensor_scalar_mul(oui[lt][0:lsz, h * 64:(h + 1) * 64],
                                                        ps[0:lsz, 64:128], rcp[0:lsz, :])

                # ---------------- iDFT + Wo ----------------
                with tc.tile_pool(name="upc", bufs=1) as upc, \
                     tc.tile_pool(name="ott", bufs=1) as ottp, \
                     tc.tile_pool(name="wop", bufs=1) as wop:
                    ur = [upc.tile([P, 512], R, tag=f"ur{t}", name=f"ur{t}") for t in range(4)]
                    ui = [upc.tile([P, 512], R, tag=f"ui{t}", name=f"ui{t}") for t in range(4)]
                    pr = [upc.tile([P, 512], R, tag=f"pr{t}", name=f"pr{t}") for t in range(4)]
                    pi = [upc.tile([P, 512], R, tag=f"pi{t}", name=f"pi{t}") for t in range(4)]
                    for t in range(4):
                        nc.vector.tensor_tensor(ur[t][:], our[t][:], our[4 + t][:], op=AL.add)
                        nc.vector.tensor_tensor(ui[t][:], oui[t][:], oui[4 + t][:], op=AL.subtract)
                        nc.vector.tensor_tensor(pr[t][:], our[t][:], our[4 + t][:], op=AL.subtract)
                        nc.vector.tensor_tensor(pi[t][:], oui[t][:], oui[4 + t][:], op=AL.add)
                    wot = [wop.tile([P, E], R, tag=f"wo{i}", name=f"wo{i}") for i in range(4)]
                    for ec in range(4):
                        nc.sync.dma_start(wot[ec][:], WoT_d.ap()[ec * P:(ec + 1) * P, :])
                    OTT = {}
                    for half in ("e", "o"):
                        OTT[half] = [ottp.tile([P, Lh], R, tag=f"OTT{half}{i}", name=f"OTT{half}{i}")
                                     for i in range(4)]
                    with tc.tile_pool(name="idps", bufs=3, space="PSUM") as idps:
                        for half, g_c, g_s, g4, c_r, c_i, e8 in (
                                ("e", gac, gas, gac4, ur, ui, our),
                                ("o", gbc, gbs, gbs4, pr, pi, oui)):
                            for fb in range(4):
                                for tck in range(2):
                                    ps = idps.tile([P, 512], F32, tag="idp", name="idp")
                                    for t in range(4):
                                        nc.tensor.matmul(ps[:], c_r[t][:, fb * P:(fb + 1) * P],
                                                         g_c[t][:, tck * 512:(tck + 1) * 512],
                                                         start=(t == 0), stop=False)
                                        nc.tensor.matmul(ps[:], c_i[t][:, fb * P:(fb + 1) * P],
                                                         g_s[t][:, tck * 512:(tck + 1) * 512],
                                                         start=False, stop=False)
                                    nc.tensor.matmul(ps[:], e8[8][0:1, fb * P:(fb + 1) * P],
                                                     g4[0:1, tck * 512:(tck + 1) * 512],
                                                     start=False, stop=True)
                                    nc.scalar.copy(OTT[half][fb][:, tck * 512:(tck + 1) * 512],
                                                   ps[:])
                    with tc.tile_pool(name="wops", bufs=2, space="PSUM") as wops, \
                         tc.tile_pool(name="ost", bufs=3) as ost:
                        for hi_, half in enumerate(("e", "o")):
                            for tb in range(8):
                                pso = [wops.tile([P, 512], F32, tag=f"po{eo}", name=f"po{eo}")
                                       for eo in range(2)]
                                for eo in range(2):
                                    for ec in range(4):
                                        nc.tensor.matmul(pso[eo][:],
                                                         OTT[half][ec][:, tb * P:(tb + 1) * P],
                                                         wot[ec][:, eo * 512:(eo + 1) * 512],
                                                         start=(ec == 0), stop=(ec == 3))
                                ot_ = ost.tile([P, E], R, tag="ot", name="ot")
                                for eo in range(2):
                                    nc.vector.tensor_copy(ot_[:, eo * 512:(eo + 1) * 512],
                                                          pso[eo][:])
                                nc.sync.dma_start(
                                    out_d.ap()[2 * P * tb + hi_:2 * P * (tb + 1):2, :], ot_[:])

    nc.finalize()
    return nc


def kernel(**inputs):
    from concourse.bass_utils import run_bass_kernel_spmd

    if "nc" not in _CACHE:
        _CACHE["nc"] = _build()
        _CACHE["consts"] = _consts()
    nc = _CACHE["nc"]
    FAc, FAs, FBc, FBs, GAc, GAs, GBc, GBs = _CACHE["consts"]

    bf = ml_dtypes.bfloat16
    q = np.asarray(inputs["query"], dtype=np.float32)
    kv = np.asarray(inputs["key_value"], dtype=np.float32)
    gamma = np.asarray(inputs["gamma"], np.float32)
    beta = np.asarray(inputs["beta"], np.float32)
    consts_bf = {
        "FAc": FAc.astype(bf), "FAs": FAs.astype(bf),
        "FBc": FBc.astype(bf), "FBs": FBs.astype(bf),
        "GAc": GAc.astype(bf), "GAs": GAs.astype(bf),
        "GBc": GBc.astype(bf), "GBs": GBs.astype(bf),
    }
    in_maps = []
    for core in range(8):
        b = core // 2
        hg = core % 2
        cs = slice(hg * 512, (hg + 1) * 512)
        m = {
            "q": np.ascontiguousarray(q[b].astype(bf)),
            "kv": np.ascontiguousarray(kv[b].astype(bf)),
            "WoT": np.ascontiguousarray(inputs["Wo"][:, cs].T.astype(bf)),
        }
        m.update(consts_bf)
        qdc = np.empty((P, NH), np.float32)
        for nm in ("qr", "qi", "kr", "ki", "vr", "vi"):
            Ws = np.asarray(inputs["W" + nm], np.float32)[cs, :]   # [512, E]
            if nm in ("qr", "qi"):
                dc = SQL * (Ws @ beta)                             # [512]
                r0 = 0 if nm == "qr" else 64
                for h in range(NH):
                    qdc[r0:r0 + 64, h] = dc[h * 64:(h + 1) * 64]
                Wk = (Ws * gamma[None, :]).T                       # [E, 512]
            else:
                Wk = Ws.T
            m[f"W{nm}"] = np.ascontiguousarray(Wk.astype(bf))
            m[f"b{nm}"] = np.ascontiguousarray(
                np.asarray(inputs["b" + nm], np.float32)[cs]).reshape(512, 1)
        m["qdc"] = qdc
        in_maps.append(m)

    res = run_bass_kernel_spmd(nc, in_maps, core_ids=list(range(8)))
    _CACHE["last"] = res
    out = np.empty((B, L, E), np.float32)
    for b in range(B):
        out[b] = (res.results[2 * b]["out"].astype(np.float32)
                  + res.results[2 * b + 1]["out"].astype(np.float32))
    return out


# revision 14
# speedup vs baseline: 1.0872x; 1.0872x over previous
"""Trainium2 Bass kernel for nn_FreqCrossAttention — radix-2 DIT version.

Sharding: 8 cores = 4 batches x 2 head-groups (8 heads each). Each core
computes a partial output [2048, 1024] (its head-group's contribution
through W_o row-parallel); host sums the pair per batch.

Freq row layout (1025 rows, used for q rows, kv rows and attention):
  row r in [0, 512)    -> freq k = r          (lo block)
  row r in [512, 1025) -> freq k = 1536 - r   (hi block, stored by m=r-512)
rfft via radix-2 DIT: A = rDFT_1024(even samples), B = twiddle-folded
rDFT_1024(odd samples); Xlo = A+B, Xhi = conj(A-B). irfft via
u = lo+conj(hi), p = lo-conj(hi) and twiddle-folded G matrices.

All matmuls bf16. Everything stays in SBUF between phases (no DRAM
round-trips), keeping the PE HAM clock-gate warm.
"""
import math
import numpy as np
import ml_dtypes

B, L, E, H = 4, 2048, 1024, 16
D = E // H            # 64
NH = 8                # heads per core
P = 128
Lh = L // 2           # 1024
Fh = Lh // 2 + 1      # 513
NF = 1025             # freq rows per side
ET = 8                # e-blocks
EPS = 1e-5
SQL = math.sqrt(L)
# freq row tiles: 8 x 128 + 1
FT = [(i * P, P) for i in range(8)] + [(1024, 1)]
# proj f-chunks: (src block, offset, size, global dst offset)
PCH = [("lo", 0, 512, 0), ("hi", 0, 256, 512), ("hi", 256, 257, 768)]
# scores f-chunks over global rows
SCH = [(0, 512), (512, 256), (768, 257)]

_CACHE = {}


def _consts():
    NORM = 1.0 / math.sqrt(L)
    n = np.arange(Lh)[:, None]
    m = np.arange(Fh)[None, :]
    ang_e = 2 * np.pi * n * m / Lh
    FAc = (np.cos(ang_e) * NORM).astype(np.float32)
    FAs = (-np.sin(ang_e) * NORM).astype(np.float32)
    ang_o = 2 * np.pi * m * (2 * n + 1) / L
    FBc = (np.cos(ang_o) * NORM).astype(np.float32)
    FBs = (-np.sin(ang_o) * NORM).astype(np.float32)

    k = np.arange(Fh)[:, None]
    nn_ = np.arange(Lh)[None, :]
    ck = np.where((k == 0) | (k == Lh // 2), 1.0, 2.0)
    th = 2 * np.pi * k * nn_ / Lh
    GAc = (ck * np.cos(th) * NORM).astype(np.float32)
    GAs = (-ck * np.sin(th) * NORM).astype(np.float32)
    ph = th + np.pi * k / Lh
    GBc = (ck * np.cos(ph) * NORM).astype(np.float32)
    GBs = (-ck * np.sin(ph) * NORM).astype(np.float32)
    GAc[512] *= 2.0   # row-512 fold: u[512] = 2*Re(lo[512]), fed as our[8]
    GBs[512] *= 2.0   # p[512] = 2i*Im(lo[512]), fed as oui[8]
    return FAc, FAs, FBc, FBs, GAc, GAs, GBc, GBs


def _build():
    import concourse.bass as bass
    import concourse.bacc as bacc
    import concourse.mybir as mybir
    import concourse.tile as tile

    R = mybir.dt.bfloat16
    F32 = mybir.dt.float32
    AF = mybir.ActivationFunctionType
    AL = mybir.AluOpType

    nc = bacc.Bacc("TRN2", debug=False, num_devices=8)

    q_d = nc.dram_tensor("q", [L, E], R, kind="ExternalInput")
    kv_d = nc.dram_tensor("kv", [L, E], R, kind="ExternalInput")
    FAc_d = nc.dram_tensor("FAc", [Lh, Fh], R, kind="ExternalInput")
    FAs_d = nc.dram_tensor("FAs", [Lh, Fh], R, kind="ExternalInput")
    FBc_d = nc.dram_tensor("FBc", [Lh, Fh], R, kind="ExternalInput")
    FBs_d = nc.dram_tensor("FBs", [Lh, Fh], R, kind="ExternalInput")
    GAc_d = nc.dram_tensor("GAc", [Fh, Lh], R, kind="ExternalInput")
    GAs_d = nc.dram_tensor("GAs", [Fh, Lh], R, kind="ExternalInput")
    GBc_d = nc.dram_tensor("GBc", [Fh, Lh], R, kind="ExternalInput")
    GBs_d = nc.dram_tensor("GBs", [Fh, Lh], R, kind="ExternalInput")
    W_d = {}
    for nm in ("qr", "qi", "kr", "ki", "vr", "vi"):
        W_d[nm] = nc.dram_tensor(f"W{nm}", [E, 512], R, kind="ExternalInput")
        W_d["b" + nm] = nc.dram_tensor(f"b{nm}", [512, 1], F32, kind="ExternalInput")
    qdc_d = nc.dram_tensor("qdc", [P, NH], F32, kind="ExternalInput")
    WoT_d = nc.dram_tensor("WoT", [512, E], R, kind="ExternalInput")
    out_d = nc.dram_tensor("out", [L, E], R, kind="ExternalOutput")

    with tile.TileContext(nc) as tc:
      with tc.tile_pool(name="persist", bufs=1) as persist:
        eps_t = persist.tile([P, 1], F32)
        nc.vector.memset(eps_t[:], EPS)
        bias_t = {}
        for nm in ("qr", "qi", "kr", "ki"):
            bias_t[nm] = []
            for mt in range(4):
                bt_ = persist.tile([P, 1], F32, tag=f"b{nm}{mt}", name=f"b{nm}{mt}")
                nc.sync.dma_start(bt_[:], W_d["b" + nm].ap()[mt * P:(mt + 1) * P, :])
                bias_t[nm].append(bt_)
        qdct = persist.tile([P, NH], F32)
        nc.sync.dma_start(qdct[:], qdc_d.ap())
        # V bias broadcast tiles
        vb_row = persist.tile([1, 512], F32)
        vbias = {}
        for nm in ("vr", "vi"):
            nc.sync.dma_start(vb_row[:], W_d["b" + nm].ap().rearrange("e one -> one e"))
            vb = persist.tile([P, 512], F32, tag=f"vb{nm}", name=f"vb{nm}")
            nc.gpsimd.partition_broadcast(vb[:], vb_row[:])
            vbias[nm] = vb

        # cat pool: attention operands (live until end of attention)
        with tc.tile_pool(name="cat", bufs=1) as catp:
            Qc = [catp.tile([P, NF], R, tag=f"Qc{h}", name=f"Qc{h}") for h in range(NH)]
            Kc = [catp.tile([P, NF], R, tag=f"Kc{h}", name=f"Kc{h}") for h in range(NH)]
            Vc = [catp.tile([P, NH * 129], R, tag=f"Vc{i}", name=f"Vc{i}") for i in range(9)]

            Wt = {}

            def dft_path(xE, xO, fa_c, fa_s, fb_c, fb_s,
                         xlo_r, xlo_i, xhi_r, xhi_i, sev, pfx):
                with tc.tile_pool(name=f"dps{pfx}", bufs=1, space="PSUM") as dps:
                    for eb in range(ET):
                        # one accumulation group per PSUM bank (start=True
                        # clears has_written for the WHOLE bank)
                        paR = dps.tile([P, 256], F32, tag="paR", name="paR")
                        paI = dps.tile([P, 256], F32, tag="paI", name="paI")
                        pa2r = dps.tile([P, 257], F32, tag="pa2r", name="pa2r")
                        pa2i = dps.tile([P, 257], F32, tag="pa2i", name="pa2i")
                        for t in range(8):
                            lhs = xE[t][:, eb * P:(eb + 1) * P]
                            st_, sp_ = (t == 0), (t == 7)
                            nc.tensor.matmul(paR[:, 0:256], lhs, fa_c[t][:, 0:256],
                                             start=st_, stop=sp_)
                            nc.tensor.matmul(pa2r[:, 0:257], lhs, fa_c[t][:, 256:513],
                                             start=st_, stop=sp_)
                            nc.tensor.matmul(paI[:, 0:256], lhs, fa_s[t][:, 0:256],
                                             start=st_, stop=sp_)
                            nc.tensor.matmul(pa2i[:, 0:257], lhs, fa_s[t][:, 256:513],
                                             start=st_, stop=sp_)
                        sAr = sev.tile([P, Fh], F32, tag="sAr", name="sAr", bufs=2)
                        sAi = sev.tile([P, Fh], F32, tag="sAi", name="sAi", bufs=2)
                        nc.scalar.copy(sAr[:, 0:256], paR[:, 0:256])
                        nc.scalar.copy(sAr[:, 256:513], pa2r[:, 0:257])
                        nc.scalar.copy(sAi[:, 0:256], paI[:, 0:256])
                        nc.scalar.copy(sAi[:, 256:513], pa2i[:, 0:257])
                        pbR = dps.tile([P, 256], F32, tag="pbR", name="pbR")
                        pbI = dps.tile([P, 256], F32, tag="pbI", name="pbI")
                        pb2r = dps.tile([P, 257], F32, tag="pb2r", name="pb2r")
                        pb2i = dps.tile([P, 257], F32, tag="pb2i", name="pb2i")
                        for t in range(8):
                            lhs = xO[t][:, eb * P:(eb + 1) * P]
                            st_, sp_ = (t == 0), (t == 7)
                            nc.tensor.matmul(pbR[:, 0:256], lhs, fb_c[t][:, 0:256],
                                             start=st_, stop=sp_)
                            nc.tensor.matmul(pb2r[:, 0:257], lhs, fb_c[t][:, 256:513],
                                             start=st_, stop=sp_)
                            nc.tensor.matmul(pbI[:, 0:256], lhs, fb_s[t][:, 0:256],
                                             start=st_, stop=sp_)
                            nc.tensor.matmul(pb2i[:, 0:257], lhs, fb_s[t][:, 256:513],
                                             start=st_, stop=sp_)
                        # combines (vector engine; one PSUM operand max)
                        nc.vector.tensor_tensor(
                            xlo_r[eb][:, 0:256], sAr[:, 0:256], pbR[:, 0:256], op=AL.add)
                        nc.vector.tensor_tensor(
                            xlo_r[eb][:, 256:512], sAr[:, 256:512], pb2r[:, 0:256], op=AL.add)
                        nc.vector.tensor_tensor(
                            xlo_i[eb][:, 0:256], sAi[:, 0:256], pbI[:, 0:256], op=AL.add)
                        nc.vector.tensor_tensor(
                            xlo_i[eb][:, 256:512], sAi[:, 256:512], pb2i[:, 0:256], op=AL.add)
                        nc.vector.tensor_tensor(
                            xhi_r[eb][:, 0:256], sAr[:, 0:256], pbR[:, 0:256], op=AL.subtract)
                        nc.vector.tensor_tensor(
                            xhi_r[eb][:, 256:513], sAr[:, 256:513], pb2r[:, 0:257], op=AL.subtract)
                        nc.vector.tensor_tensor(
                            xhi_i[eb][:, 0:256], pbI[:, 0:256], sAi[:, 0:256], op=AL.subtract)
                        nc.vector.tensor_tensor(
                            xhi_i[eb][:, 256:513], pb2i[:, 0:257], sAi[:, 256:513], op=AL.subtract)

            def qk_proj(xlo_r, xlo_i, xhi_r, xhi_i, nmr, nmi, cat, qdc_fix):
                with tc.tile_pool(name="pps", bufs=1, space="PSUM") as pps, \
                     tc.tile_pool(name="stg", bufs=3) as stg:
                    for (src, c0, csz, g0) in PCH:
                        xr = xlo_r if src == "lo" else xhi_r
                        xi = xlo_i if src == "lo" else xhi_i
                        for mt in range(4):
                            ppr = pps.tile([P, 512], F32, tag="ppr", name="ppr", bufs=2)
                            ppi = pps.tile([P, 512], F32, tag="ppi", name="ppi", bufs=2)
                            for ec in range(ET):
                                st_, sp_ = (ec == 0), (ec == ET - 1)
                                nc.tensor.matmul(ppr[:, 0:csz],
                                                 Wt[nmr][ec][:, mt * P:(mt + 1) * P],
                                                 xr[ec][:, c0:c0 + csz],
                                                 start=st_, stop=sp_)
                                nc.tensor.matmul(ppi[:, 0:csz],
                                                 Wt[nmi][ec][:, mt * P:(mt + 1) * P],
                                                 xi[ec][:, c0:c0 + csz],
                                                 start=st_, stop=sp_)
                            sgr = stg.tile([P, 512], R, tag="sgr", name="sgr")
                            sgi = stg.tile([P, 512], R, tag="sgi", name="sgi")
                            nc.scalar.activation(sgr[:, 0:csz], ppr[:, 0:csz],
                                                 AF.Identity, bias=bias_t[nmr][mt][:])
                            nc.scalar.activation(sgi[:, 0:csz], ppi[:, 0:csz],
                                                 AF.Identity, bias=bias_t[nmi][mt][:])
                            h0, h1 = 2 * mt, 2 * mt + 1
                            nc.sync.dma_start(cat[h0][0:64, g0:g0 + csz], sgr[0:64, 0:csz])
                            nc.sync.dma_start(cat[h1][0:64, g0:g0 + csz], sgr[64:128, 0:csz])
                            nc.sync.dma_start(cat[h0][64:128, g0:g0 + csz], sgi[0:64, 0:csz])
                            nc.sync.dma_start(cat[h1][64:128, g0:g0 + csz], sgi[64:128, 0:csz])
                    if qdc_fix:
                        for h in range(NH):
                            nc.vector.tensor_tensor(cat[h][:, 0:1], cat[h][:, 0:1],
                                                    qdct[:, h:h + 1], op=AL.add)

            def v_proj(xlo_r, xlo_i, xhi_r, xhi_i):
                with tc.tile_pool(name="vps", bufs=1, space="PSUM") as vps:
                    for mi, (m0, msz) in enumerate(FT):
                        if mi < 4:
                            xr = [xlo_r[ec][:, m0:m0 + msz] for ec in range(ET)]
                            xi = [xlo_i[ec][:, m0:m0 + msz] for ec in range(ET)]
                        else:
                            c0 = m0 - 512
                            xr = [xhi_r[ec][:, c0:c0 + msz] for ec in range(ET)]
                            xi = [xhi_i[ec][:, c0:c0 + msz] for ec in range(ET)]
                        pvr = vps.tile([P, 512], F32, tag="pvr", name="pvr", bufs=2)
                        pvi = vps.tile([P, 512], F32, tag="pvi", name="pvi", bufs=2)
                        for ec in range(ET):
                            st_, sp_ = (ec == 0), (ec == ET - 1)
                            nc.tensor.matmul(pvr[0:msz, :], xr[ec], Wt["vr"][ec][:],
                                             start=st_, stop=sp_)
                            nc.tensor.matmul(pvi[0:msz, :], xi[ec], Wt["vi"][ec][:],
                                             start=st_, stop=sp_)
                        vco = Vc[mi][0:msz, :].rearrange("p (h c) -> p h c", h=NH)
                        nc.vector.tensor_add(
                            vco[:, :, 0:64],
                            pvr[0:msz, :].rearrange("p (h c) -> p h c", h=NH),
                            vbias["vr"][0:msz, :].rearrange("p (h c) -> p h c", h=NH))
                        nc.vector.tensor_add(
                            vco[:, :, 64:128],
                            pvi[0:msz, :].rearrange("p (h c) -> p h c", h=NH),
                            vbias["vi"][0:msz, :].rearrange("p (h c) -> p h c", h=NH))
                        nc.vector.memset(vco[:, :, 128:129], 1.0)

            # ---------------- DFT + projection phases ----------------
            with tc.tile_pool(name="fmat", bufs=1) as fm, \
                 tc.tile_pool(name="qn", bufs=1) as qnp:
                qnE = [qnp.tile([P, E], R, tag=f"qnE{t}", name=f"qnE{t}") for t in range(8)]
                qnO = [qnp.tile([P, E], R, tag=f"qnO{t}", name=f"qnO{t}") for t in range(8)]
                fa_c, fa_s, fb_c, fb_s = [], [], [], []

                # ---- kv path ----
                with tc.tile_pool(name="xkv", bufs=1) as xkv:
                    kXloR = [xkv.tile([P, 512], R, tag=f"kXloR{e}", name=f"kXloR{e}") for e in range(ET)]
                    kXloI = [xkv.tile([P, 512], R, tag=f"kXloI{e}", name=f"kXloI{e}") for e in range(ET)]
                    kXhiR = [xkv.tile([P, Fh], R, tag=f"kXhiR{e}", name=f"kXhiR{e}") for e in range(ET)]
                    kXhiI = [xkv.tile([P, Fh], R, tag=f"kXhiI{e}", name=f"kXhiI{e}") for e in range(ET)]
                    with tc.tile_pool(name="kvio", bufs=1) as kvio:
                        kvE, kvO = [], []
                        # issue in dependency-useful order: kvE+FAc+FAs first
                        for t in range(8):
                            kt = kvio.tile([P, E], R, tag=f"kvE{t}", name=f"kvE{t}")
                            nc.sync.dma_start(kt[:], kv_d.ap()[2 * P * t:2 * P * (t + 1):2, :])
                            kvE.append(kt)
                            fc = fm.tile([P, Fh], R, tag=f"fac{t}", name=f"fac{t}")
                            nc.sync.dma_start(fc[:], FAc_d.ap()[P * t:P * (t + 1), :])
                            fa_c.append(fc)
                            fs = fm.tile([P, Fh], R, tag=f"fas{t}", name=f"fas{t}")
                            nc.sync.dma_start(fs[:], FAs_d.ap()[P * t:P * (t + 1), :])
                            fa_s.append(fs)
                        for t in range(8):
                            kt = kvio.tile([P, E], R, tag=f"kvO{t}", name=f"kvO{t}")
                            nc.sync.dma_start(kt[:], kv_d.ap()[2 * P * t + 1:2 * P * (t + 1):2, :])
                            kvO.append(kt)
                            fc = fm.tile([P, Fh], R, tag=f"fbc{t}", name=f"fbc{t}")
                            nc.sync.dma_start(fc[:], FBc_d.ap()[P * t:P * (t + 1), :])
                            fb_c.append(fc)
                            fs = fm.tile([P, Fh], R, tag=f"fbs{t}", name=f"fbs{t}")
                            nc.sync.dma_start(fs[:], FBs_d.ap()[P * t:P * (t + 1), :])
                            fb_s.append(fs)

                        # LN of q -> qnE/qnO (scalar+vector, overlaps kv DFT)
                        with tc.tile_pool(name="lnq", bufs=1) as lnio, \
                             tc.tile_pool(name="lns", bufs=4) as lns:
                            for t in range(16):
                                par = t % 2
                                tt = t // 2
                                qt = lnio.tile([P, E], R, tag="qt", name="qt", bufs=3)
                                nc.sync.dma_start(
                                    qt[:], q_d.ap()[2 * P * tt + par:2 * P * (tt + 1):2, :])
                                st = lns.tile([P, 12], F32, tag="st", name="st")
                                nc.vector.bn_stats(st[:, 0:6], qt[:, 0:512])
                                nc.vector.bn_stats(st[:, 6:12], qt[:, 512:1024])
                                mv = lns.tile([P, 2], F32, tag="mv", name="mv")
                                nc.vector.bn_aggr(mv[:], st[:])
                                sd = lns.tile([P, 1], F32, tag="sd", name="sd")
                                nc.scalar.activation(sd[:], mv[:, 1:2], AF.Sqrt, bias=eps_t[:])
                                istd = lns.tile([P, 1], F32, tag="istd", name="istd")
                                nc.vector.reciprocal(istd[:], sd[:])
                                nmu = lns.tile([P, 1], F32, tag="nmu", name="nmu")
                                nc.vector.tensor_scalar_mul(nmu[:], mv[:, 0:1], -1.0)
                                nc.vector.tensor_mul(nmu[:], nmu[:], istd[:])
                                dst = qnE[tt] if par == 0 else qnO[tt]
                                nc.scalar.activation(dst[:], qt[:], AF.Identity,
                                                     bias=nmu[:], scale=istd[:])

                        dft_path(kvE, kvO, fa_c, fa_s, fb_c, fb_s,
                                 kXloR, kXloI, kXhiR, kXhiI, xkv, "kv")
                    # kvio closed: kvE/kvO + q-in freed
                    with tc.tile_pool(name="wk", bufs=1) as wk:
                        for nm in ("kr", "ki"):
                            Wt[nm] = []
                            for ec in range(ET):
                                w = wk.tile([P, 512], R, tag=f"W{nm}{ec}", name=f"W{nm}{ec}")
                                nc.sync.dma_start(w[:], W_d[nm].ap()[ec * P:(ec + 1) * P, :])
                                Wt[nm].append(w)
                        qk_proj(kXloR, kXloI, kXhiR, kXhiI, "kr", "ki", Kc, False)
                    with tc.tile_pool(name="wv", bufs=1) as wv:
                        for nm in ("vr", "vi"):
                            Wt[nm] = []
                            for ec in range(ET):
                                w = wv.tile([P, 512], R, tag=f"W{nm}{ec}", name=f"W{nm}{ec}")
                                nc.sync.dma_start(w[:], W_d[nm].ap()[ec * P:(ec + 1) * P, :])
                                Wt[nm].append(w)
                        v_proj(kXloR, kXloI, kXhiR, kXhiI)

                # ---- q path ---- (xkv closed)
                with tc.tile_pool(name="xq", bufs=1) as xq:
                    qXloR = [xq.tile([P, 512], R, tag=f"qXloR{e}", name=f"qXloR{e}") for e in range(ET)]
                    qXloI = [xq.tile([P, 512], R, tag=f"qXloI{e}", name=f"qXloI{e}") for e in range(ET)]
                    qXhiR = [xq.tile([P, Fh], R, tag=f"qXhiR{e}", name=f"qXhiR{e}") for e in range(ET)]
                    qXhiI = [xq.tile([P, Fh], R, tag=f"qXhiI{e}", name=f"qXhiI{e}") for e in range(ET)]
                    dft_path(qnE, qnO, fa_c, fa_s, fb_c, fb_s,
                             qXloR, qXloI, qXhiR, qXhiI, xq, "q")
                    with tc.tile_pool(name="wq", bufs=1) as wq:
                        for nm in ("qr", "qi"):
                            Wt[nm] = []
                            for ec in range(ET):
                                w = wq.tile([P, 512], R, tag=f"W{nm}{ec}", name=f"W{nm}{ec}")
                                nc.sync.dma_start(w[:], W_d[nm].ap()[ec * P:(ec + 1) * P, :])
                                Wt[nm].append(w)
                        qk_proj(qXloR, qXloI, qXhiR, qXhiI, "qr", "qi", Qc, True)

            # ---------------- attention ---------------- (fm/qn closed)
            with tc.tile_pool(name="oacc", bufs=1) as oacc, \
                 tc.tile_pool(name="gmat", bufs=1) as gm:
                our = [oacc.tile([P, 512], R, tag=f"our{i}", name=f"our{i}") for i in range(9)]
                oui = [oacc.tile([P, 512], R, tag=f"oui{i}", name=f"oui{i}") for i in range(9)]
                # preload iDFT matrices during attention
                gac = [gm.tile([P, Lh], R, tag=f"gac{t}", name=f"gac{t}") for t in range(4)]
                gas = [gm.tile([P, Lh], R, tag=f"gas{t}", name=f"gas{t}") for t in range(4)]
                gbc = [gm.tile([P, Lh], R, tag=f"gbc{t}", name=f"gbc{t}") for t in range(4)]
                gbs = [gm.tile([P, Lh], R, tag=f"gbs{t}", name=f"gbs{t}") for t in range(4)]
                gac4 = gm.tile([1, Lh], R, tag="gac4", name="gac4")
                gbs4 = gm.tile([1, Lh], R, tag="gbs4", name="gbs4")
                for t in range(4):
                    nc.sync.dma_start(gac[t][:], GAc_d.ap()[t * P:(t + 1) * P, :])
                    nc.sync.dma_start(gas[t][:], GAs_d.ap()[t * P:(t + 1) * P, :])
                    nc.sync.dma_start(gbc[t][:], GBc_d.ap()[t * P:(t + 1) * P, :])
                    nc.sync.dma_start(gbs[t][:], GBs_d.ap()[t * P:(t + 1) * P, :])
                nc.sync.dma_start(gac4[:], GAc_d.ap()[512:513, :])
                nc.sync.dma_start(gbs4[:], GBs_d.ap()[512:513, :])

                with tc.tile_pool(name="expp", bufs=3) as expp, \
                     tc.tile_pool(name="sps", bufs=2, space="PSUM") as sps, \
                     tc.tile_pool(name="avps", bufs=2, space="PSUM") as avps, \
                     tc.tile_pool(name="nrm", bufs=4) as nrm:
                    for h in range(NH):
                        expts = []
                        for mi, (m0, msz) in enumerate(FT):
                            et_ = expp.tile([P, NF], R, tag=f"exp{mi}", name=f"exp{mi}")
                            ps = sps.tile([P, 1536], F32, tag="sc", name="sc")
                            for (f0, fsz) in ((0, 512), (512, 512), (1024, 1)):
                                nc.tensor.matmul(ps[0:msz, f0:f0 + fsz], Kc[h][:, m0:m0 + msz],
                                                 Qc[h][:, f0:f0 + fsz], start=True, stop=True)
                            nc.scalar.activation(et_[0:msz, 0:NF], ps[0:msz, 0:NF],
                                                 AF.Exp, scale=float(D ** -0.5))
                            expts.append(et_)
                        for lt, (l0, lsz) in enumerate(FT):
                            ps = avps.tile([P, 129], F32, tag="av", name="av")
                            for mi, (m0, msz) in enumerate(FT):
                                nc.tensor.matmul(ps[0:lsz, :], expts[mi][0:msz, l0:l0 + lsz],
                                                 Vc[mi][0:msz, h * 129:(h + 1) * 129],
                                                 start=(mi == 0), stop=(mi == 8))
                            rcp = nrm.tile([P, 1], F32, tag="rcp", name="rcp")
                            nc.vector.reciprocal(rcp[0:lsz, :], ps[0:lsz, 128:129])
                            nc.vector.tensor_scalar_mul(our[lt][0:lsz, h * 64:(h + 1) * 64],
                                                        ps[0:lsz, 0:64], rcp[0:lsz, :])
                            nc.vector.tensor_scalar_mul(oui[lt][0:lsz, h * 64:(h + 1) * 64],
                                                        ps[0:lsz, 64:128], rcp[0:lsz, :])

                # ---------------- iDFT + Wo ----------------
                with tc.tile_pool(name="upc", bufs=1) as upc, \
                     tc.tile_pool(name="ott", bufs=1) as ottp, \
                     tc.tile_pool(name="wop", bufs=1) as wop:
                    ur = [upc.tile([P, 512], R, tag=f"ur{t}", name=f"ur{t}") for t in range(4)]
                    ui = [upc.tile([P, 512], R, tag=f"ui{t}", name=f"ui{t}") for t in range(4)]
                    pr = [upc.tile([P, 512], R, tag=f"pr{t}", name=f"pr{t}") for t in range(4)]
                    pi = [upc.tile([P, 512], R, tag=f"pi{t}", name=f"pi{t}") for t in range(4)]
                    for t in range(4):
                        nc.vector.tensor_tensor(ur[t][:], our[t][:], our[4 + t][:], op=AL.add)
                        nc.vector.tensor_tensor(ui[t][:], oui[t][:], oui[4 + t][:], op=AL.subtract)
                        nc.vector.tensor_tensor(pr[t][:], our[t][:], our[4 + t][:], op=AL.subtract)
                        nc.vector.tensor_tensor(pi[t][:], oui[t][:], oui[4 + t][:], op=AL.add)
                    wot = [wop.tile([P, E], R, tag=f"wo{i}", name=f"wo{i}") for i in range(4)]
                    for ec in range(4):
                        nc.sync.dma_start(wot[ec][:], WoT_d.ap()[ec * P:(ec + 1) * P, :])
                    OTT = {}
                    for half in ("e", "o"):
                        OTT[half] = [ottp.tile([P, Lh], R, tag=f"OTT{half}{i}", name=f"OTT{half}{i}")
                                     for i in range(4)]
                    with tc.tile_pool(name="idps", bufs=3, space="PSUM") as idps:
                        for half, g_c, g_s, g4, c_r, c_i, e8 in (
                                ("e", gac, gas, gac4, ur, ui, our),
                                ("o", gbc, gbs, gbs4, pr, pi, oui)):
                            for fb in range(4):
                                for tck in range(2):
                                    ps = idps.tile([P, 512], F32, tag="idp", name="idp")
                                    for t in range(4):
                                        nc.tensor.matmul(ps[:], c_r[t][:, fb * P:(fb + 1) * P],
                                                         g_c[t][:, tck * 512:(tck + 1) * 512],
                                                         start=(t == 0), stop=False)
                                        nc.tensor.matmul(ps[:], c_i[t][:, fb * P:(fb + 1) * P],
                                                         g_s[t][:, tck * 512:(tck + 1) * 512],
                                                         start=False, stop=False)
                                    nc.tensor.matmul(ps[:], e8[8][0:1, fb * P:(fb + 1) * P],
                                                     g4[0:1, tck * 512:(tck + 1) * 512],
                                                     start=False, stop=True)
                                    nc.scalar.copy(OTT[half][fb][:, tck * 512:(tck + 1) * 512],
                                                   ps[:])
                    with tc.tile_pool(name="wops", bufs=2, space="PSUM") as wops, \
                         tc.tile_pool(name="ost", bufs=3) as ost:
                        for hi_, half in enumerate(("e", "o")):
                            for tb in range(8):
                                pso = [wops.tile([P, 512], F32, tag=f"po{eo}", name=f"po{eo}")
                                       for eo in range(2)]
                                for eo in range(2):
                                    for ec in range(4):
                                        nc.tensor.matmul(pso[eo][:],
                                                         OTT[half][ec][:, tb * P:(tb + 1) * P],
                                                         wot[ec][:, eo * 512:(eo + 1) * 512],
                                                         start=(ec == 0), stop=(ec == 3))
                                ot_ = ost.tile([P, E], R, tag="ot", name="ot")
                                for eo in range(2):
                                    nc.vector.tensor_copy(ot_[:, eo * 512:(eo + 1) * 512],
                                                          pso[eo][:])
                                nc.sync.dma_start(
                                    out_d.ap()[2 * P * tb + hi_:2 * P * (tb + 1):2, :], ot_[:])

    nc.finalize()
    return nc


def kernel(**inputs):
    from concourse.bass_utils import run_bass_kernel_spmd

    if "nc" not in _CACHE:
        _CACHE["nc"] = _build()
        _CACHE["consts"] = _consts()
    nc = _CACHE["nc"]
    FAc, FAs, FBc, FBs, GAc, GAs, GBc, GBs = _CACHE["consts"]

    bf = ml_dtypes.bfloat16
    q = np.asarray(inputs["query"], dtype=np.float32)
    kv = np.asarray(inputs["key_value"], dtype=np.float32)
    gamma = np.asarray(inputs["gamma"], np.float32)
    beta = np.asarray(inputs["beta"], np.float32)
    consts_bf = {
        "FAc": FAc.astype(bf), "FAs": FAs.astype(bf),
        "FBc": FBc.astype(bf), "FBs": FBs.astype(bf),
        "GAc": GAc.astype(bf), "GAs": GAs.astype(bf),
        "GBc": GBc.astype(bf), "GBs": GBs.astype(bf),
    }
    in_maps = []
    for core in range(8):
        b = core // 2
        hg = core % 2
        cs = slice(hg * 512, (hg + 1) * 512)
        m = {
            "q": np.ascontiguousarray(q[b].astype(bf)),
            "kv": np.ascontiguousarray(kv[b].astype(bf)),
            "WoT": np.ascontiguousarray(inputs["Wo"][:, cs].T.astype(bf)),
        }
        m.update(consts_bf)
        qdc = np.empty((P, NH), np.float32)
        for nm in ("qr", "qi", "kr", "ki", "vr", "vi"):
            Ws = np.asarray(inputs["W" + nm], np.float32)[cs, :]   # [512, E]
            if nm in ("qr", "qi"):
                dc = SQL * (Ws @ beta)                             # [512]
                r0 = 0 if nm == "qr" else 64
                for h in range(NH):
                    qdc[r0:r0 + 64, h] = dc[h * 64:(h + 1) * 64]
                Wk = (Ws * gamma[None, :]).T                       # [E, 512]
            else:
                Wk = Ws.T
            m[f"W{nm}"] = np.ascontiguousarray(Wk.astype(bf))
            m[f"b{nm}"] = np.ascontiguousarray(
                np.asarray(inputs["b" + nm], np.float32)[cs]).reshape(512, 1)
        m["qdc"] = qdc
        in_maps.append(m)

    res = run_bass_kernel_spmd(nc, in_maps, core_ids=list(range(8)))
    _CACHE["last"] = res
    out = np.empty((B, L, E), np.float32)
    for b in range(B):
        out[b] = (res.results[2 * b]["out"].astype(np.float32)
                  + res.results[2 * b + 1]["out"].astype(np.float32))
    return out
